# revision 1
# baseline (speedup 1.0000x reference)
"""Trainium2 Bass kernel for a BiQRNN3D layer.

reference math:
  gates = conv3d(x, W, SAME, 3x3x3) + b          x: [2,16,31,256,256] f32
  Z, F1, F2 = split(gates, 3, channel)           W: [48,16,3,3,3], b: [48]
  Z = tanh(Z); F1 = sigmoid(F1); F2 = sigmoid(F2)
  h_fwd: depth-forward  recurrence h = F1*h + (1-F1)*Z
  h_bwd: depth-backward recurrence h = F2*h + (1-F2)*Z
  out = h_fwd + h_bwd                            [2,16,31,256,256] f32

Distribution: H (=256) is sharded 32 rows per core across 8 NeuronCores
(SPMD, identical program; each core's x shard carries its 1-row conv halo
with global-edge zeros baked in by the host).

Per-core pipeline:
  * conv as matmul, K = (kd,ci) = 48 contraction rows. The moving x tile
    holds 3 kd-shifted copies in partitions 0-47 (block A) and an
    additional h+1-shifted copy in partitions 64-111 (block B,
    host-prepared). Partition 48 is a ones-row (bias rides as a stationary
    row); partitions 49-63 are zeros.
  * M = 96: stationary columns (j, co) produce BOTH output h rows of an
    h-block at once. Per psum tile [96, 2*256] six K=112 matmuls
    accumulate: passes (p in {0,1}) x (kw in {0,1,2}); pass p streams x
    rows at tile-h 2p, and blocks A/B provide taps kh = 2p-j and 2p+1-j.
  * gates spill to DRAM fp16 [48, D, S]; XBAR DMA-transpose returns
    128-pixel chunks as [128, (co,d)].
  * ACT: tanh/sigmoid at 128-partition utilization; DVE: g = (f-1)*z,
    tensor_tensor_scan (h = f*h - g) for both directions (backward stored
    d-reversed); f zeroed at d=0 so one long scan chains safely across
    channel runs. out fp32 [S, 16, 31] -> host reassembles.
"""

from contextlib import ExitStack

import numpy as np

import concourse.bass as bass
import concourse.tile as tile
from concourse import bacc, mybir

F32 = mybir.dt.float32
F16 = mybir.dt.float16
AF = mybir.ActivationFunctionType
ALU = mybir.AluOpType

N_CORES = 8
B = 2
CIN = 16
HID = 16
CO = 3 * HID            # 48
D = 31
H = 256
W = 256
HSH = H // N_CORES      # 32
HB = 2                  # output h rows per conv tile (= M/CO)
DC = 2                  # d slices per psum tile
WP = W + 2
S = B * HSH * W         # 16384
FX = D * 2 * WP         # x tile free extent per partition
CHUNK = 128
NST = 6                 # stationary matrices


def _build_program(reps=1, do_conv=True, do_scan=True, do_evac=True,
                   do_spill=True, fake_tp=False):
    nc = bacc.Bacc("TRN2", target_bir_lowering=False, debug=False)

    x_dram = nc.dram_tensor("x", [CIN, D + 2, B, HSH + 2, WP], F16,
                            kind="ExternalInput").ap()
    wts = nc.dram_tensor("wts", [128, NST * 2 * CO], F16,
                         kind="ExternalInput").ap()
    aux = nc.dram_tensor("aux", [16, FX], F16, kind="ExternalInput").ap()
    gates = nc.dram_tensor("gates", [CO, D, S], F16, kind="Internal").ap()
    out = nc.dram_tensor("out", [S, HID, D], F32, kind="ExternalOutput").ap()

    with tile.TileContext(nc) as tc, ExitStack() as ctx:
        wsb = nc.alloc_sbuf_tensor("wsb", [128, NST * 2 * CO], F16).ap()
        # x tile: A rows hold x at h = h0 + 2t, B rows x at h0 + 1 + 2t
        xbufs = [nc.alloc_sbuf_tensor(f"xb{i}", [112, D, 2, WP], F16).ap()
                 for i in range(2)]

        nc.sync.dma_start(wsb, wts)
        for xb in xbufs:
            nc.sync.dma_start(
                xb[48:64].rearrange("p a b c -> p (a b c)"), aux)

        ev_pool = ctx.enter_context(tc.tile_pool(name="ev", bufs=2))
        ps_pool = ctx.enter_context(tc.tile_pool(name="ps", bufs=4,
                                                 space="PSUM"))
        t_pool = ctx.enter_context(tc.tile_pool(name="tp", bufs=8))
        sc_pool = ctx.enter_context(tc.tile_pool(name="sc", bufs=4))

        n_hblk = HSH // HB
        n_dc = (D + DC - 1) // DC
        gates2d = gates.rearrange("c d s -> (c d) s")
        CD = CO * D

        chunk_q = []
        per_blk = (HB * W) // CHUNK

        def scan_block(c0s):
            if not do_scan:
                return
            Ts = []
            for c0 in c0s:
                T = t_pool.tile([128, CD], F16, tag="T")
                if fake_tp:
                    nc.sync.dma_start(T[:], gates2d[0:128, 0:CD])
                else:
                    nc.sync.dma_start(T[:], gates2d[:, c0:c0 + CHUNK],
                                      transpose=True)
                Ts.append(T)
            for c0, T in zip(c0s, Ts):
                scan_chunk(c0, T)

        def scan_chunk(c0, T):
            Tv = T[:].rearrange("p (c d) -> p c d", d=D)
            zt = sc_pool.tile([128, HID, D], F16, tag="zt")
            f1 = sc_pool.tile([128, HID, D], F16, tag="f1")
            f2 = sc_pool.tile([128, HID, D], F16, tag="f2")
            nc.scalar.activation(zt[:], Tv[:, 0:HID], AF.Tanh)
            nc.scalar.activation(f1[:], Tv[:, HID:2 * HID], AF.Sigmoid)
            nc.scalar.activation(f2[:, :, ::-1], Tv[:, 2 * HID:3 * HID],
                                 AF.Sigmoid)
            g1 = sc_pool.tile([128, HID, D], F16, tag="g1")
            g2 = sc_pool.tile([128, HID, D], F16, tag="g2")
            nc.vector.scalar_tensor_tensor(
                g1[:], f1[:], 1.0, zt[:], ALU.subtract, ALU.mult)
            nc.vector.scalar_tensor_tensor(
                g2[:], f2[:], 1.0, zt[:, :, ::-1], ALU.subtract, ALU.mult)
            nc.vector.memset(f1[:, :, 0:1], 0.0)
            nc.vector.memset(f2[:, :, 0:1], 0.0)
            h1 = sc_pool.tile([128, HID, D], F32, tag="h1")
            h2 = sc_pool.tile([128, HID, D], F32, tag="h2")
            nc.vector.tensor_tensor_scan(
                h1[:].rearrange("p c d -> p (c d)"),
                f1[:].rearrange("p c d -> p (c d)"),
                g1[:].rearrange("p c d -> p (c d)"),
                0.0, ALU.mult, ALU.subtract)
            nc.vector.tensor_tensor_scan(
                h2[:].rearrange("p c d -> p (c d)"),
                f2[:].rearrange("p c d -> p (c d)"),
                g2[:].rearrange("p c d -> p (c d)"),
                0.0, ALU.mult, ALU.subtract)
            o = sc_pool.tile([128, HID, D], F32, tag="o")
            nc.vector.tensor_add(o[:], h1[:], h2[:, :, ::-1])
            nc.gpsimd.dma_start(out[c0:c0 + CHUNK], o[:])

        tix = 0
        for _rep in range(reps):
            for b_i in range(B):
                for hb_i in range(n_hblk):
                    xb = xbufs[tix % 2]
                    tix += 1
                    h0 = hb_i * HB
                    for kd in range(3):
                        for t in range(2):
                            nc.sync.dma_start(
                                xb[kd * 16:kd * 16 + 16, :, t],
                                x_dram[:, kd:kd + D, b_i, h0 + 2 * t, :])
                            nc.sync.dma_start(
                                xb[64 + kd * 16:64 + kd * 16 + 16, :, t],
                                x_dram[:, kd:kd + D, b_i, h0 + 1 + 2 * t, :])
                    s0 = b_i * (HSH * W) + h0 * W
                    for dc in range(n_dc if do_conv else 0):
                        d0 = dc * DC
                        dn = min(DC, D - d0)
                        ps = ps_pool.tile([2 * CO, DC * W], F32, tag="ps")
                        psv = ps[:, 0:dn * W].rearrange(
                            "p (d w) -> p d w", w=W)
                        k = 0
                        for p in range(2):
                            for kw in range(3):
                                nc.tensor.matmul(
                                    psv,
                                    wsb[0:112, k * 96:(k + 1) * 96],
                                    xb[0:112, d0:d0 + dn, p, kw:kw + W],
                                    start=(k == 0), stop=(k == NST - 1))
                                k += 1
                        if not do_evac:
                            continue
                        ev = ev_pool.tile([2 * CO, DC * W], F16, tag="ev")
                        evv = ev[:, 0:dn * W]
                        if dc % 2 == 0:
                            nc.scalar.copy(evv, ps[:, 0:dn * W])
                        else:
                            nc.vector.tensor_copy(evv, ps[:, 0:dn * W])
                        if not do_spill:
                            continue
                        for j in range(2):
                            nc.scalar.dma_start(
                                gates[:, d0:d0 + dn,
                                      s0 + j * W:s0 + (j + 1) * W],
                                ev[j * CO:(j + 1) * CO, 0:dn * W].rearrange(
                                    "p (d w) -> p d w", w=W))
                    chunk_q.append([s0 + kq * CHUNK
                                    for kq in range(per_blk)])
                    while len(chunk_q) > 1:
                        scan_block(chunk_q.pop(0))
        while chunk_q:
            scan_block(chunk_q.pop(0))

    nc.finalize()
    return nc


def _host_inputs(x, Wc, b):
    """x: [B, CIN, D, H, W] f32 full input. Returns list of 8 in_maps."""
    bf = np.float16
    # 6 stationaries: idx = p*3+kw, each [128, 96] with cols (j*48+co).
    # rows 0-47 (block A, x at tile-h 2p):   tap kh = 2p - j
    # rows 64-111 (block B, x at h+1):       tap kh = 2p + 1 - j
    wt = np.zeros((NST, 128, 2 * CO), np.float32)
    for p in range(2):
        for kw in range(3):
            idx = p * 3 + kw
            for j in range(2):
                c0 = j * CO
                for blk, khv in ((0, 2 * p - j), (64, 2 * p + 1 - j)):
                    if khv < 0 or khv > 2:
                        continue
                    for kd in range(3):
                        p0 = blk + kd * 16
                        wt[idx, p0:p0 + 16, c0:c0 + CO] = \
                            Wc[:, :, kd, khv, kw].T
    wt[0, 48, 0:CO] = b
    wt[0, 48, CO:2 * CO] = b
    wts = wt.transpose(1, 0, 2).reshape(128, NST * 2 * CO).astype(bf)
    auxa = np.zeros((16, FX), np.float32)
    auxa[0, :] = 1.0
    auxa = auxa.astype(bf)

    xt = np.ascontiguousarray(x.transpose(1, 2, 0, 3, 4))  # [CIN,D,B,H,W]
    in_maps = []
    for c in range(N_CORES):
        hs, he = c * HSH, (c + 1) * HSH
        xp = np.zeros((CIN, D + 2, B, HSH + 2, WP), np.float32)
        lo = max(hs - 1, 0)
        hi = min(he + 1, H)
        xp[:, 1:D + 1, :, (lo - (hs - 1)):(hi - (hs - 1)), 1:W + 1] = \
            xt[:, :, :, lo:hi, :]
        in_maps.append({"x": xp.astype(bf), "wts": wts, "aux": auxa})
    return in_maps


_PROGRAM = None


def _get_program():
    global _PROGRAM
    if _PROGRAM is None:
        _PROGRAM = _build_program()
    return _PROGRAM


def run_sharded(in_maps, trace=False, **kw):
    from concourse import bass_utils
    nc = _get_program()
    return bass_utils.run_bass_kernel_spmd(
        nc, in_maps, core_ids=list(range(N_CORES)), trace=trace, **kw)


def _assemble(results):
    outf = np.empty((B, HID, D, H, W), np.float32)
    for c in range(N_CORES):
        raw = np.asarray(results[c]["out"])  # [S, HID, D]
        o = raw.reshape(B, HSH, W, HID, D).transpose(0, 3, 4, 1, 2)
        outf[:, :, :, c * HSH:(c + 1) * HSH, :] = o
    return outf


def kernel(x, W, b):
    x = np.asarray(x, np.float32)
    W = np.asarray(W, np.float32)
    b = np.asarray(b, np.float32)
    in_maps = _host_inputs(x, W, b)
    res = run_sharded(in_maps)
    return _assemble(res.results)



# revision 7
# speedup vs baseline: 12899.0013x; 12899.0013x over previous
"""Trainium2 Bass kernel for a BiQRNN3D layer.

reference math:
  gates = conv3d(x, W, SAME, 3x3x3) + b          x: [2,16,31,256,256] f32
  Z, F1, F2 = split(gates, 3, channel)           W: [48,16,3,3,3], b: [48]
  Z = tanh(Z); F1 = sigmoid(F1); F2 = sigmoid(F2)
  h_fwd: depth-forward  recurrence h = F1*h + (1-F1)*Z
  h_bwd: depth-backward recurrence h = F2*h + (1-F2)*Z
  out = h_fwd + h_bwd                            [2,16,31,256,256] f32

Distribution: H (=256) is sharded 32 rows per core across 8 NeuronCores
(SPMD, identical program; each core's x shard carries its 1-row conv halo
with global-edge zeros baked in by the host).

Per-core pipeline (v2 — no DRAM gates round-trip):
  * conv as matmul, K=(kd,ci)=48 rows per h-copy. x tile partitions 0-47
    hold 3 kd-shifted copies at h rows h0+2t (block A), partitions 64-111
    at h0+1+2t (block B); partition 48 = ones (bias row), 49-63 zeros.
    Host stores x as [B, 34h, CIN, 33d, 258w] fp16 so each x-tile
    partition loads as ONE contiguous 16KB DMA run.
  * M=96: psum [2h x 48co, 2d x 256w]; 6 K=112 matmuls (p x kw) per tile.
  * psum evacuates (fp32->fp16 cast) into SBUF G[96, 31d, 256w].
  * TensorEngine transposes G[96, 128w-chunk] -> PSUM fp16 [128, 96],
    8 d-slices packed per psum bank; ACT (tanh/sigmoid, reading PSUM)
    writes scan-layout tiles [128pix, 16co, 31d], F2 d-reversed.
  * GpSimd: g = (f-1)*z; DVE: tensor_tensor_scan (h = f*h - g) both
    directions; o = h1 + rev(h2) -> fp16; DMA out [S, HID, D] fp16.
"""

from contextlib import ExitStack

import numpy as np

import concourse.bass as bass
import concourse.tile as tile
from concourse import bacc, mybir
from concourse.masks import make_identity

F32 = mybir.dt.float32
F16 = mybir.dt.float16
AF = mybir.ActivationFunctionType
ALU = mybir.AluOpType

N_CORES = 8
B = 2
CIN = 16
HID = 16
CO = 3 * HID            # 48
D = 31
H = 256
W = 256
HSH = H // N_CORES      # 32
HB = 2                  # output h rows per conv block
DC = 2                  # d slices per psum tile
DP = D + 2              # 33
WP = W + 2              # 258
S = B * HSH * W         # 16384
CHUNK = 128
NST = 6                 # stationary matrices
DG = 8                  # d slices per transpose psum group
NBLK = B * (HSH // HB)  # 32


def _build_program():
    nc = bacc.Bacc("TRN2", target_bir_lowering=False, debug=False)

    x_dram = nc.dram_tensor("x", [B, HSH + 2, CIN, DP, WP], F16,
                            kind="ExternalInput").ap()
    wts = nc.dram_tensor("wts", [128, NST * 2 * CO], F16,
                         kind="ExternalInput").ap()
    aux = nc.dram_tensor("aux", [16, 2 * D * WP], F16,
                         kind="ExternalInput").ap()
    out = nc.dram_tensor("out", [S, HID, D], F16, kind="ExternalOutput").ap()

    n_hblk = HSH // HB

    with tile.TileContext(nc) as tc, ExitStack() as ctx:
        wsb = nc.alloc_sbuf_tensor("wsb", [128, NST * 2 * CO], F16).ap()
        ident = nc.alloc_sbuf_tensor("ident", [128, 128], F16).ap()
        # x tile: [partition, t, d, w]; A rows (0-47) h=h0+2t, B rows
        # (64-111) h=h0+1+2t; row 48 ones (bias), 49-63 zeros.
        xbufs = [nc.alloc_sbuf_tensor(f"xb{i}", [112, 2, D, WP], F16).ap()
                 for i in range(2)]

        nc.sync.dma_start(wsb, wts)
        make_identity(nc, ident)
        for xb in xbufs:
            nc.sync.dma_start(
                xb[48:64].rearrange("p a b c -> p (a b c)"), aux)

        ps_pool = ctx.enter_context(tc.tile_pool(name="ps", bufs=4,
                                                 space="PSUM"))
        tp_pool = ctx.enter_context(tc.tile_pool(name="tp", bufs=3,
                                                 space="PSUM"))
        g_pool = ctx.enter_context(tc.tile_pool(name="gp", bufs=2))
        sc_pool = ctx.enter_context(tc.tile_pool(name="sc", bufs=6))

        n_dc = (D + DC - 1) // DC

        def load_x(k):
            b_i, hb_i = divmod(k, n_hblk)
            xb = xbufs[k % 2]
            h0 = hb_i * HB
            for kd in range(3):
                for t in range(2):
                    nc.sync.dma_start(
                        xb[kd * 16:kd * 16 + 16, t],
                        x_dram[b_i, h0 + 2 * t, :, kd:kd + D, :])
                    nc.sync.dma_start(
                        xb[64 + kd * 16:64 + kd * 16 + 16, t],
                        x_dram[b_i, h0 + 1 + 2 * t, :, kd:kd + D, :])

        def conv_block(k):
            xb = xbufs[k % 2]
            G = g_pool.tile([96, D, W], F16, tag="G")
            for dc in range(n_dc):
                d0 = dc * DC
                dn = min(DC, D - d0)
                ps = ps_pool.tile([2 * CO, DC * W], F32, tag="ps")
                psv = ps[:, 0:dn * W].rearrange("p (d w) -> p d w", w=W)
                kk = 0
                for p in range(2):
                    for kw in range(3):
                        nc.tensor.matmul(
                            psv,
                            wsb[0:112, kk * 96:(kk + 1) * 96],
                            xb[0:112, p, d0:d0 + dn, kw:kw + W],
                            start=(kk == 0), stop=(kk == NST - 1))
                        kk += 1
                if dc % 2 == 0:
                    nc.scalar.copy(G[:, d0:d0 + dn, :], psv)
                else:
                    nc.vector.tensor_copy(G[:, d0:d0 + dn, :], psv)
            return G

        def scan_chunk(c0, zt, f1, f2):
            g1 = sc_pool.tile([128, HID, D], F16, tag="g1")
            g2 = sc_pool.tile([128, HID, D], F16, tag="g2")
            nc.vector.scalar_tensor_tensor(
                g1[:], f1[:], 1.0, zt[:], ALU.subtract, ALU.mult)
            nc.vector.scalar_tensor_tensor(
                g2[:], f2[:], 1.0, zt[:, :, ::-1], ALU.subtract, ALU.mult)
            nc.vector.memset(f1[:, :, 0:1], 0.0)
            nc.vector.memset(f2[:, :, 0:1], 0.0)
            h1 = sc_pool.tile([128, HID, D], F32, tag="h1")
            h2 = sc_pool.tile([128, HID, D], F32, tag="h2")
            nc.vector.tensor_tensor_scan(
                h1[:].rearrange("p c d -> p (c d)"),
                f1[:].rearrange("p c d -> p (c d)"),
                g1[:].rearrange("p c d -> p (c d)"),
                0.0, ALU.mult, ALU.subtract)
            nc.vector.tensor_tensor_scan(
                h2[:].rearrange("p c d -> p (c d)"),
                f2[:].rearrange("p c d -> p (c d)"),
                g2[:].rearrange("p c d -> p (c d)"),
                0.0, ALU.mult, ALU.subtract)
            o = sc_pool.tile([128, HID, D], F16, tag="o")
            nc.vector.tensor_add(o[:], h1[:], h2[:, :, ::-1])
            nc.sync.dma_start(out[c0:c0 + CHUNK], o[:])

        def process_block(k, G):
            b_i, hb_i = divmod(k, n_hblk)
            h0 = hb_i * HB
            s0 = b_i * (HSH * W) + h0 * W
            for wc in range(2):
                zts, f1s, f2s = [], [], []
                for j in range(2):
                    zt = sc_pool.tile([128, HID, D], F16, tag=f"zt{j}")
                    f1 = sc_pool.tile([128, HID, D], F16, tag=f"f1{j}")
                    f2 = sc_pool.tile([128, HID, D], F16, tag=f"f2{j}")
                    zts.append(zt)
                    f1s.append(f1)
                    f2s.append(f2)
                for dg in range(0, D, DG):
                    dn_g = min(DG, D - dg)
                    P = tp_pool.tile([128, DG * 96], F16, tag="P")
                    for i in range(dn_g):
                        nc.tensor.transpose(
                            P[:, i * 96:(i + 1) * 96],
                            G[0:96, dg + i, wc * 128:(wc + 1) * 128],
                            ident[0:96, 0:96])
                    Pv = P[:].rearrange("p (dd jc) -> p jc dd", jc=96)
                    Pv = Pv[:, :, 0:dn_g]
                    for j in range(2):
                        c0 = j * 48
                        nc.scalar.activation(
                            zts[j][:, :, dg:dg + dn_g],
                            Pv[:, c0:c0 + 16, :], AF.Tanh)
                        nc.scalar.activation(
                            f1s[j][:, :, dg:dg + dn_g],
                            Pv[:, c0 + 16:c0 + 32, :], AF.Sigmoid)
                        nc.scalar.activation(
                            f2s[j][:, :, D - dg - dn_g:D - dg][:, :, ::-1],
                            Pv[:, c0 + 32:c0 + 48, :], AF.Sigmoid)
                for j in range(2):
                    scan_chunk(s0 + j * W + wc * 128, zts[j], f1s[j], f2s[j])

        load_x(0)
        Gs = {}
        for k in range(NBLK):
            if k + 1 < NBLK:
                load_x(k + 1)
            Gs[k] = conv_block(k)
            if k - 1 in Gs:
                process_block(k - 1, Gs.pop(k - 1))
        process_block(NBLK - 1, Gs.pop(NBLK - 1))

    nc.finalize()
    return nc


def _host_inputs(x, Wc, b):
    """x: [B, CIN, D, H, W] f32 full input. Returns list of 8 in_maps."""
    bf = np.float16
    # 6 stationaries: idx = p*3+kw, each [128, 96] with cols (j*48+co).
    # rows 0-47 (block A, x at tile-h 2p):   tap kh = 2p - j
    # rows 64-111 (block B, x at h+1):       tap kh = 2p + 1 - j
    wt = np.zeros((NST, 128, 2 * CO), np.float32)
    for p in range(2):
        for kw in range(3):
            idx = p * 3 + kw
            for j in range(2):
                c0 = j * CO
                for blk, khv in ((0, 2 * p - j), (64, 2 * p + 1 - j)):
                    if khv < 0 or khv > 2:
                        continue
                    for kd in range(3):
                        p0 = blk + kd * 16
                        wt[idx, p0:p0 + 16, c0:c0 + CO] = \
                            Wc[:, :, kd, khv, kw].T
    wt[0, 48, 0:CO] = b
    wt[0, 48, CO:2 * CO] = b
    wts = wt.transpose(1, 0, 2).reshape(128, NST * 2 * CO).astype(bf)
    auxa = np.zeros((16, 2 * D * WP), np.float32)
    auxa[0, :] = 1.0
    auxa = auxa.astype(bf)

    # x layout: [B, 34h', CIN, 33d, 258w]; h'=0 is global row hs-1 (halo),
    # d index dd = x_d + 1, w index = x_w + 1; edges zero.
    xt = np.ascontiguousarray(x.transpose(0, 3, 1, 2, 4))  # [B,H,CIN,D,W]
    in_maps = []
    for c in range(N_CORES):
        hs, he = c * HSH, (c + 1) * HSH
        xp = np.zeros((B, HSH + 2, CIN, DP, WP), np.float32)
        lo = max(hs - 1, 0)
        hi = min(he + 1, H)
        xp[:, (lo - (hs - 1)):(hi - (hs - 1)), :, 1:D + 1, 1:W + 1] = \
            xt[:, lo:hi, :, :, :]
        in_maps.append({"x": xp.astype(bf), "wts": wts, "aux": auxa})
    return in_maps


_PROGRAM = None


def _get_program():
    global _PROGRAM
    if _PROGRAM is None:
        _PROGRAM = _build_program()
    return _PROGRAM


def run_sharded(in_maps, trace=False, **kw):
    from concourse import bass_utils
    nc = _get_program()
    return bass_utils.run_bass_kernel_spmd(
        nc, in_maps, core_ids=list(range(N_CORES)), trace=trace, **kw)


def _assemble(results):
    outf = np.empty((B, HID, D, H, W), np.float32)
    for c in range(N_CORES):
        raw = np.asarray(results[c]["out"]).astype(np.float32)  # [S, HID, D]
        o = raw.reshape(B, HSH, W, HID, D).transpose(0, 3, 4, 1, 2)
        outf[:, :, :, c * HSH:(c + 1) * HSH, :] = o
    return outf


def kernel(x, W, b):
    x = np.asarray(x, np.float32)
    W = np.asarray(W, np.float32)
    b = np.asarray(b, np.float32)
    in_maps = _host_inputs(x, W, b)
    res = run_sharded(in_maps)
    return _assemble(res.results)
